# revision 7
# baseline (speedup 1.0000x reference)
"""Cross-attention kernel for Trainium2, 8 NeuronCores.

Reference computation (per batch b, with n = h*w = 9216, c = 128, cq = 16):
    q  = (w_q @ y_b)                       # [cq, n]   (used transposed)
    k  = (w_k @ y_b)                       # [cq, n]
    s  = q^T @ k                           # [n, n]    scores
    m  = softmax(s, axis=-1)
    v  = (w_v @ x_b)                       # [c, n]
    out = v @ m^T                          # [c, n]

Sharding: 8 cores = (batch b in {0,1}) x (query block qb in {0..3}, 2304
queries each). Each core sees all 9216 keys.

v2 design (engine-balanced flash loop), changes vs the v1 baseline:
- Score matmuls run as 3-way tile_position volleys (strips 0/32/64) instead
  of v1's 2-way: st tiles are [128, 3, 512] so each concurrent matmul owns a
  full 2KB PSUM bank.  (4-way volleys are infeasible: concurrently active
  packed matmuls faulting the same PSUM bank crash the hardware -- verified
  empirically -- and 4 banks x 2 slots + feat + aux exceeds the 8 banks.)
- The softmax denominator is mostly OFF the PE: exp tiles (fp16) are
  accumulated elementwise on the DVE (2x 2-byte mode) into a per-window acc
  tile; only the last XPE groups of each window keep PE ones-matmuls, which
  balances PE vs DVE occupancy.  (In v1 the den ones-matmuls cost as much
  PE time as the feat matmuls.)  The acc tile is folded into the same aux
  PSUM accumulator by ones-matmuls at window end, so recip/mul stay in the
  simple v1 [128, qwd] form.
- exp output is fp16 (2x ACT throughput, ~4x less rounding error than
  bf16), making the feat/den moving operands fp16 (1 cycle/col at any
  width).  VT is fp16 too (walrus forbids mixing 32-bit with 16-bit matmul
  inputs).  Measured error ~6e-4 in CoreSim.
- VT prep uses fp16 wv as the moving operand: 128 cols x 1 cyc/col, 4x
  cheaper than v1's plain-f32 matmuls.
- f32->f32r/fp16 rounds run on ACT, PSUM evacuations on DVE, splitting the
  prep work across both engines.
- PSUM budget exactly 8 banks: 2 st slots x 3 + feat + aux.
"""

import numpy as np

import concourse.bacc as bacc
import concourse.tile as tile
from concourse import mybir

f32 = mybir.dt.float32
f32r = mybir.dt.float32r
fp16 = mybir.dt.float16

P = 128          # partitions / channels
NK = 9216        # keys (h*w)
NQ = 2304        # queries per core
KC = NK // P     # 72 key chunks of 128
CQ = 16          # query/key projection dim
# Query windows: four of 512 plus a 256 tail (256 keeps the fp32r fast path)
W_SPANS = [(0, 512), (512, 512), (1024, 512), (1536, 512), (2048, 256)]
GSZ = 3          # key chunks per group (3-way tile_position volley)
NG = KC // GSZ   # 24 groups per window
LAG = 3          # groups of software-pipelining lag for feat/den
XPE = 3          # groups per window whose denominator runs on the PE

_CACHE = {}


def _build():
    nc = bacc.Bacc(trn_type="TRN2", target_bir_lowering=False, debug=False)
    y = nc.dram_tensor("y", [P, NK], f32, kind="ExternalInput")
    yq = nc.dram_tensor("yq", [P, NQ], f32, kind="ExternalInput")
    x = nc.dram_tensor("x", [P, NK], f32, kind="ExternalInput")
    # w_q^T / w_k^T replicated into four 32-row strips ([wT,0,wT,0,wT,0,wT])
    # so the score matmuls can run 4-way row-packed via tile_position.
    wq = nc.dram_tensor("wq", [P, 112], f32, kind="ExternalInput")
    wk = nc.dram_tensor("wk", [P, 112], f32, kind="ExternalInput")
    wv = nc.dram_tensor("wv", [P, P], f32, kind="ExternalInput")    # w_v^T
    o = nc.dram_tensor("o", [P, NQ], f32, kind="ExternalOutput")

    Exp = mybir.ActivationFunctionType.Exp
    Copy = mybir.ActivationFunctionType.Copy

    with tile.TileContext(nc) as tc:
        with (
            tc.tile_pool(name="const", bufs=1) as const,
            tc.tile_pool(name="big", bufs=1) as big,
            tc.tile_pool(name="xs", bufs=2) as xs,
            tc.tile_pool(name="ps", bufs=2, space="PSUM") as ps,
            tc.tile_pool(name="featp", bufs=1, space="PSUM") as featp,
            tc.tile_pool(name="auxp", bufs=1, space="PSUM") as auxp,
            tc.tile_pool(name="ep", bufs=LAG + 2) as ep,
            tc.tile_pool(name="accp", bufs=2) as accp,
            tc.tile_pool(name="op", bufs=2) as op,
            tc.tile_pool(name="small", bufs=2) as small,
        ):
            # ---- constants ----
            wq_sb = const.tile([P, 112], f32, name="wq_sb")
            nc.sync.dma_start(wq_sb, wq.ap())
            wk_sb = const.tile([P, 112], f32, name="wk_sb")
            nc.sync.dma_start(wk_sb, wk.ap())
            wv_sb = const.tile([P, P], f32, name="wv_sb")
            nc.sync.dma_start(wv_sb, wv.ap())
            wvb = const.tile([P, P], fp16, name="wvb")
            nc.vector.tensor_copy(wvb, wv_sb)
            wkr = const.tile([P, 112], f32r, name="wkr")
            nc.vector.tensor_copy(wkr, wk_sb)
            wqr = const.tile([P, 112], f32r, name="wqr")
            nc.vector.tensor_copy(wqr, wq_sb)
            ones_st = const.tile([P, P], f32, name="ones_st")
            nc.vector.memset(ones_st, 1.0)
            ones_big = const.tile([P, P], fp16, name="ones_big")
            nc.vector.tensor_copy(ones_big, ones_st)

            K_sb = big.tile([112, NK], f32r, name="K_sb")
            Q_sb = big.tile([112, NQ], f32r, name="Q_sb")
            VT = big.tile([P, NK], fp16, name="VT")

            # ---- prep ----
            # yq first (the whole Q projection gates the first score matmul),
            # then y/x chunks interleaved.  f32->f32r rounds go on ACT, the
            # PSUM evacuations on DVE.
            def emit_proj(i):
                src = y.ap()[:, i * NQ : (i + 1) * NQ] if i < 4 else yq.ap()
                yst = xs.tile([P, NQ], f32, tag="yst", name=f"yst{i}")
                nc.sync.dma_start(yst, src)
                yr = xs.tile([P, NQ], f32r, tag="yr", name=f"yr{i}")
                nc.scalar.activation(yr, yst, Copy)
                wr = wkr if i < 4 else wqr
                dst = K_sb if i < 4 else Q_sb
                dof = i * NQ if i < 4 else 0
                for t, qs in enumerate(range(0, NQ, 512)):
                    qw = min(512, NQ - qs)
                    kp = ps.tile([112, qw], f32, tag="st", name=f"kp{i}_{t}")
                    nc.tensor.matmul(kp, wr, yr[:, qs : qs + qw], start=True, stop=True)
                    nc.vector.tensor_copy(dst[:, dof + qs : dof + qs + qw], kp)

            def emit_vt(i):
                # vT chunks [128 keys, 128 c] = x_chunk^T @ w_v^T with fp16
                # moving wv (1 cyc/col); evacuate eight chunks per DVE copy.
                xt = xs.tile([P, NQ], f32, tag="xt", name=f"xt{i}")
                nc.sync.dma_start(xt, x.ap()[:, i * NQ : (i + 1) * NQ])
                xr = xs.tile([P, NQ], fp16, tag="xr", name=f"xr{i}")
                nc.scalar.activation(xr, xt, Copy)
                nkc = NQ // P  # 18
                for b0 in range(0, nkc, 8):
                    nb = min(8, nkc - b0)
                    vp = ps.tile([P, nb * P], f32, tag="st", name=f"vp{i}_{b0}")
                    for t in range(b0, b0 + nb):
                        nc.tensor.matmul(
                            vp[:, (t - b0) * P : (t - b0 + 1) * P],
                            xr[:, t * P : (t + 1) * P],
                            wvb,
                            start=True,
                            stop=True,
                        )
                    kc0 = i * nkc + b0
                    nc.vector.tensor_copy(VT[:, kc0 * P : (kc0 + nb) * P], vp)

            emit_proj(4)  # yq -> Q_sb
            for i in range(4):
                emit_proj(i)
                emit_vt(i)

            # ---- main flash loop, software-pipelined ----
            # The PE engine queue is in-order: feat/den matmuls are emitted
            # LAG groups behind their score matmuls so the PE never waits on
            # the exp of the group it just scored.
            groups = [
                (wi, ws, qwd, g)
                for wi, (ws, qwd) in enumerate(W_SPANS)
                for g in range(NG)
            ]
            et_tiles = {}
            acc_tiles = {}
            feat_tiles = {}
            aux_tiles = {}

            def emit_st(wi, ws, qwd, g):
                # each of the GSZ concurrent volley matmuls owns a full 2KB
                # PSUM bank (512 f32 cols padded), required by hardware
                st = ps.tile(
                    [P, GSZ, 512], f32, tag="st", name=f"st{wi}_{g}"
                )
                for j in range(GSZ):
                    kc = GSZ * g + j
                    nc.tensor.matmul(
                        st[:, j, :qwd],
                        K_sb[32 * j : 32 * j + CQ, kc * P : (kc + 1) * P],
                        Q_sb[32 * j : 32 * j + CQ, ws : ws + qwd],
                        start=True,
                        stop=True,
                        tile_position=(32 * j, 0),
                    )
                et = ep.tile([P, GSZ, qwd], fp16, tag="e", name=f"e{wi}_{g}")
                nc.scalar.activation(et, st[:, :, :qwd], Exp)
                et_tiles[(wi, g)] = et
                # DVE side of the denominator: elementwise accumulate the exp
                # tiles (2x fp16 DVE mode); the last XPE groups are left for
                # PE ones-matmuls at feat time to balance the two engines.
                if g < NG - XPE:
                    if g == 0:
                        acc = accp.tile(
                            [P, GSZ, qwd], fp16, tag="acc", name=f"acc{wi}"
                        )
                        acc_tiles[wi] = acc
                        nc.vector.tensor_copy(acc, et)
                    else:
                        nc.vector.tensor_add(acc_tiles[wi], acc_tiles[wi], et)

            def emit_fd(wi, ws, qwd, g):
                if g == 0:
                    feat_tiles[wi] = featp.tile(
                        [P, qwd], f32, tag="feat", name=f"feat{wi}",
                        padded_shape=[P, 512],
                    )
                    aux_tiles[wi] = auxp.tile(
                        [P, qwd], f32, tag="aux", name=f"aux{wi}",
                        padded_shape=[P, 512],
                    )
                feat_ps = feat_tiles[wi]
                aux = aux_tiles[wi]
                et = et_tiles.pop((wi, g))
                for j in range(GSZ):
                    kc = GSZ * g + j
                    nc.tensor.matmul(
                        feat_ps,
                        VT[:, kc * P : (kc + 1) * P],
                        et[:, j, :],
                        start=(kc == 0),
                        stop=(kc == KC - 1),
                    )
                if g >= NG - XPE:
                    # PE share of the denominator (all 128 output rows carry
                    # the same key-sum, v1-style)
                    for j in range(GSZ):
                        nc.tensor.matmul(
                            aux,
                            ones_big,
                            et[:, j, :],
                            start=(g == NG - XPE and j == 0),
                            stop=False,
                        )
                if g == NG - 1:
                    acc = acc_tiles.pop(wi)
                    for j in range(GSZ):
                        nc.tensor.matmul(
                            aux,
                            ones_big,
                            acc[:, j, :],
                            start=False,
                            stop=(j == GSZ - 1),
                        )
                    aux_tiles.pop(wi)
                    rec = small.tile([P, qwd], f32, tag="rec", name=f"rec{wi}")
                    nc.vector.reciprocal(rec, aux)
                    o_sb = op.tile([P, qwd], f32, tag="o", name=f"o{wi}")
                    nc.vector.tensor_mul(o_sb, feat_tiles.pop(wi), rec)
                    nc.sync.dma_start(o.ap()[:, ws : ws + qwd], o_sb)

            for idx in range(len(groups) + LAG):
                if idx < len(groups):
                    emit_st(*groups[idx])
                if idx >= LAG:
                    emit_fd(*groups[idx - LAG])

    nc.compile()
    return nc


def _get_runner():
    """Build the Bass module once and wrap it in a cached sharded jax callable.

    Mirrors concourse.bass2jax.run_bass_via_pjrt (the @via_axon execution
    path) but caches the jitted executable so repeated kernel() calls do not
    re-trace/re-compile.
    """
    if "runner" in _CACHE:
        return _CACHE["runner"]

    import jax
    from jax.experimental.shard_map import shard_map
    from jax.sharding import Mesh, PartitionSpec

    from concourse import bass2jax, mybir as _mybir

    bass2jax.install_neuronx_cc_hook()
    nc = _build()

    partition_name = nc.partition_id_tensor.name if nc.partition_id_tensor else None
    in_names, out_names, out_avals = [], [], []
    for alloc in nc.m.functions[0].allocations:
        if not isinstance(alloc, _mybir.MemoryLocationSet):
            continue
        name = alloc.memorylocations[0].name
        if alloc.kind == "ExternalInput":
            if name != partition_name:
                in_names.append(name)
        elif alloc.kind == "ExternalOutput":
            out_names.append(name)
            out_avals.append(
                jax.core.ShapedArray(
                    tuple(alloc.tensor_shape), _mybir.dt.np(alloc.dtype)
                )
            )
    n_params = len(in_names)
    all_in_names = in_names + out_names
    if partition_name is not None:
        all_in_names.append(partition_name)
    donate = tuple(range(n_params, n_params + len(out_names)))

    def _body(*args):
        operands = list(args)
        if partition_name is not None:
            operands.append(bass2jax.partition_id_tensor())
        outs = bass2jax._bass_exec_p.bind(
            *operands,
            out_avals=tuple(out_avals),
            in_names=tuple(all_in_names),
            out_names=tuple(out_names),
            lowering_input_output_aliases=(),
            sim_require_finite=True,
            sim_require_nnan=True,
            nc=nc,
        )
        return tuple(outs)

    devices = jax.devices()[:8]
    mesh = Mesh(np.asarray(devices), ("core",))
    in_specs = (PartitionSpec("core"),) * (n_params + len(out_names))
    out_specs = (PartitionSpec("core"),) * len(out_names)
    smapped = shard_map(
        _body, mesh=mesh, in_specs=in_specs, out_specs=out_specs, check_rep=False
    )
    sharded = jax.jit(smapped, donate_argnums=donate, keep_unused=True)

    out_shapes = [tuple(a.shape) for a in out_avals]
    out_dtypes = [a.dtype for a in out_avals]
    runner = {
        "fn": sharded,
        "smapped": smapped,
        "n_params": n_params,
        "in_names": in_names,
        "out_names": out_names,
        "out_shapes": out_shapes,
        "out_dtypes": out_dtypes,
        "nc": nc,
    }
    _CACHE["runner"] = runner
    return runner


def _run(in_maps):
    r = _get_runner()
    concat_in = [
        np.concatenate([np.asarray(m[name]) for m in in_maps], axis=0)
        for name in r["in_names"]
    ]
    concat_zeros = [
        np.zeros((8 * s[0], *s[1:]), d)
        for s, d in zip(r["out_shapes"], r["out_dtypes"])
    ]
    out_arrs = r["fn"](*concat_in, *concat_zeros)
    return [
        {
            name: np.asarray(out_arrs[i]).reshape(8, *r["out_shapes"][i])[c]
            for i, name in enumerate(r["out_names"])
        }
        for c in range(8)
    ]


def _make_in_maps(x, y, w_q, w_k, w_v):
    x = np.ascontiguousarray(np.asarray(x, dtype=np.float32))
    y = np.ascontiguousarray(np.asarray(y, dtype=np.float32))
    bz, c, h, w = x.shape
    n = h * w
    xf = x.reshape(bz, c, n)
    yf = y.reshape(bz, c, n)
    wqT = np.asarray(w_q, dtype=np.float32).T  # [c, cq]
    wkT = np.asarray(w_k, dtype=np.float32).T
    z = np.zeros((c, 32 - CQ), np.float32)
    wq2 = np.ascontiguousarray(
        np.concatenate([wqT, z, wqT, z, wqT, z, wqT], axis=1)
    )  # [c, 112]
    wk2 = np.ascontiguousarray(np.concatenate([wkT, z, wkT, z, wkT, z, wkT], axis=1))
    wvT = np.ascontiguousarray(np.asarray(w_v, dtype=np.float32).T)  # [c, c]
    in_maps = []
    for cid in range(8):
        b, qb = divmod(cid, 4)
        in_maps.append(
            {
                "y": np.ascontiguousarray(yf[b]),
                "yq": np.ascontiguousarray(yf[b][:, qb * NQ : (qb + 1) * NQ]),
                "x": np.ascontiguousarray(xf[b]),
                "wq": wq2,
                "wk": wk2,
                "wv": wvT,
            }
        )
    return in_maps


def kernel(x, y, w_q, w_k, w_v):
    bz, c, h, w = np.asarray(x).shape
    n = h * w
    results = _run(_make_in_maps(x, y, w_q, w_k, w_v))
    feat = np.empty((bz, c, n), dtype=np.float32)
    for cid in range(8):
        b, qb = divmod(cid, 4)
        feat[b][:, qb * NQ : (qb + 1) * NQ] = results[cid]["o"]
    return feat.reshape(bz, c, h, w)


# revision 9
# speedup vs baseline: 1.2251x; 1.2251x over previous
"""Cross-attention kernel for Trainium2, 8 NeuronCores.

Reference computation (per batch b, with n = h*w = 9216, c = 128, cq = 16):
    q  = (w_q @ y_b)                       # [cq, n]   (used transposed)
    k  = (w_k @ y_b)                       # [cq, n]
    s  = q^T @ k                           # [n, n]    scores
    m  = softmax(s, axis=-1)
    v  = (w_v @ x_b)                       # [c, n]
    out = v @ m^T                          # [c, n]

Sharding: 8 cores = (batch b in {0,1}) x (query block qb in {0..3}, 2304
queries each). Each core sees all 9216 keys.

v2 design (engine-balanced flash loop), changes vs the v1 baseline:
- Score matmuls run as 3-way tile_position volleys (strips 0/32/64) instead
  of v1's 2-way: st tiles are [128, 3, 512] so each concurrent matmul owns a
  full 2KB PSUM bank.  (4-way volleys are infeasible: concurrently active
  packed matmuls faulting the same PSUM bank crash the hardware -- verified
  empirically -- and 4 banks x 2 slots + feat + aux exceeds the 8 banks.)
- The softmax denominator is mostly OFF the PE: exp tiles (fp16) are
  accumulated elementwise on the DVE (2x 2-byte mode) into a per-window acc
  tile; only the last XPE groups of each window keep PE ones-matmuls, which
  balances PE vs DVE occupancy.  (In v1 the den ones-matmuls cost as much
  PE time as the feat matmuls.)  The acc tile is folded into the same aux
  PSUM accumulator by ones-matmuls at window end, so recip/mul stay in the
  simple v1 [128, qwd] form.
- exp output is fp16 (2x ACT throughput, ~4x less rounding error than
  bf16), making the feat/den moving operands fp16 (1 cycle/col at any
  width).  VT is fp16 too (walrus forbids mixing 32-bit with 16-bit matmul
  inputs).  Measured error ~6e-4 in CoreSim.
- VT prep uses fp16 wv as the moving operand: 128 cols x 1 cyc/col, 4x
  cheaper than v1's plain-f32 matmuls.
- f32->f32r/fp16 rounds run on ACT, PSUM evacuations on DVE, splitting the
  prep work across both engines.
- PSUM budget exactly 8 banks: 2 st slots x 3 + feat + aux.
"""

import numpy as np

import concourse.bacc as bacc
import concourse.tile as tile
from concourse import mybir

f32 = mybir.dt.float32
f32r = mybir.dt.float32r
fp16 = mybir.dt.float16

P = 128          # partitions / channels
NK = 9216        # keys (h*w)
NQ = 2304        # queries per core
KC = NK // P     # 72 key chunks of 128
CQ = 16          # query/key projection dim
# Query windows: four of 512 plus a 256 tail (256 keeps the fp32r fast path)
W_SPANS = [(0, 512), (512, 512), (1024, 512), (1536, 512), (2048, 256)]
GSZ = 3          # key chunks per group (3-way tile_position volley)
NG = KC // GSZ   # 24 groups per window
LAG = 3          # groups of software-pipelining lag for feat/den
XPE = 0          # groups per window whose denominator runs on the PE

_CACHE = {}


def _build(xpe=None, lag=None):
    xpe = XPE if xpe is None else xpe
    lag = LAG if lag is None else lag
    nc = bacc.Bacc(trn_type="TRN2", target_bir_lowering=False, debug=False)
    y = nc.dram_tensor("y", [P, NK], f32, kind="ExternalInput")
    yq = nc.dram_tensor("yq", [P, NQ], f32, kind="ExternalInput")
    x = nc.dram_tensor("x", [P, NK], f32, kind="ExternalInput")
    # w_q^T / w_k^T replicated into four 32-row strips ([wT,0,wT,0,wT,0,wT])
    # so the score matmuls can run 4-way row-packed via tile_position.
    wq = nc.dram_tensor("wq", [P, 112], f32, kind="ExternalInput")
    wk = nc.dram_tensor("wk", [P, 112], f32, kind="ExternalInput")
    wv = nc.dram_tensor("wv", [P, P], f32, kind="ExternalInput")    # w_v^T
    o = nc.dram_tensor("o", [P, NQ], f32, kind="ExternalOutput")

    Exp = mybir.ActivationFunctionType.Exp
    Copy = mybir.ActivationFunctionType.Copy

    with tile.TileContext(nc) as tc:
        with (
            tc.tile_pool(name="const", bufs=1) as const,
            tc.tile_pool(name="big", bufs=1) as big,
            tc.tile_pool(name="xs", bufs=2) as xs,
            tc.tile_pool(name="ps", bufs=2, space="PSUM") as ps,
            tc.tile_pool(name="featp", bufs=1, space="PSUM") as featp,
            tc.tile_pool(name="auxp", bufs=1, space="PSUM") as auxp,
            tc.tile_pool(name="ep", bufs=lag + 2) as ep,
            tc.tile_pool(name="accp", bufs=2) as accp,
            tc.tile_pool(name="op", bufs=2) as op,
            tc.tile_pool(name="small", bufs=2) as small,
        ):
            # ---- constants ----
            wq_sb = const.tile([P, 112], f32, name="wq_sb")
            nc.sync.dma_start(wq_sb, wq.ap())
            wk_sb = const.tile([P, 112], f32, name="wk_sb")
            nc.sync.dma_start(wk_sb, wk.ap())
            wv_sb = const.tile([P, P], f32, name="wv_sb")
            nc.sync.dma_start(wv_sb, wv.ap())
            wvb = const.tile([P, P], fp16, name="wvb")
            nc.vector.tensor_copy(wvb, wv_sb)
            wkr = const.tile([P, 112], f32r, name="wkr")
            nc.vector.tensor_copy(wkr, wk_sb)
            wqr = const.tile([P, 112], f32r, name="wqr")
            nc.vector.tensor_copy(wqr, wq_sb)
            ones_st = const.tile([P, P], f32, name="ones_st")
            nc.vector.memset(ones_st, 1.0)
            ones_big = const.tile([P, P], fp16, name="ones_big")
            nc.vector.tensor_copy(ones_big, ones_st)

            K_sb = big.tile([112, NK], f32r, name="K_sb")
            Q_sb = big.tile([112, NQ], f32r, name="Q_sb")
            VT = big.tile([P, NK], fp16, name="VT")

            # ---- prep ----
            # yq first (the whole Q projection gates the first score matmul),
            # then y/x chunks interleaved.  f32->f32r rounds go on ACT, the
            # PSUM evacuations on DVE.
            def emit_proj(i):
                src = y.ap()[:, i * NQ : (i + 1) * NQ] if i < 4 else yq.ap()
                yst = xs.tile([P, NQ], f32, tag="yst", name=f"yst{i}")
                nc.sync.dma_start(yst, src)
                yr = xs.tile([P, NQ], f32r, tag="yr", name=f"yr{i}")
                nc.scalar.activation(yr, yst, Copy)
                wr = wkr if i < 4 else wqr
                dst = K_sb if i < 4 else Q_sb
                dof = i * NQ if i < 4 else 0
                for t, qs in enumerate(range(0, NQ, 512)):
                    qw = min(512, NQ - qs)
                    kp = ps.tile([112, qw], f32, tag="st", name=f"kp{i}_{t}")
                    nc.tensor.matmul(kp, wr, yr[:, qs : qs + qw], start=True, stop=True)
                    nc.vector.tensor_copy(dst[:, dof + qs : dof + qs + qw], kp)

            def emit_vt(i):
                # vT chunks [128 keys, 128 c] = x_chunk^T @ w_v^T with fp16
                # moving wv (1 cyc/col); evacuate eight chunks per DVE copy.
                xt = xs.tile([P, NQ], f32, tag="xt", name=f"xt{i}")
                nc.sync.dma_start(xt, x.ap()[:, i * NQ : (i + 1) * NQ])
                xr = xs.tile([P, NQ], fp16, tag="xr", name=f"xr{i}")
                nc.scalar.activation(xr, xt, Copy)
                nkc = NQ // P  # 18
                for b0 in range(0, nkc, 8):
                    nb = min(8, nkc - b0)
                    vp = ps.tile([P, nb * P], f32, tag="st", name=f"vp{i}_{b0}")
                    for t in range(b0, b0 + nb):
                        nc.tensor.matmul(
                            vp[:, (t - b0) * P : (t - b0 + 1) * P],
                            xr[:, t * P : (t + 1) * P],
                            wvb,
                            start=True,
                            stop=True,
                        )
                    kc0 = i * nkc + b0
                    nc.vector.tensor_copy(VT[:, kc0 * P : (kc0 + nb) * P], vp)

            emit_proj(4)  # yq -> Q_sb
            for i in range(4):
                emit_proj(i)
                emit_vt(i)

            # ---- main flash loop, software-pipelined ----
            # The PE engine queue is in-order: feat/den matmuls are emitted
            # LAG groups behind their score matmuls so the PE never waits on
            # the exp of the group it just scored.
            groups = [
                (wi, ws, qwd, g)
                for wi, (ws, qwd) in enumerate(W_SPANS)
                for g in range(NG)
            ]
            et_tiles = {}
            acc_tiles = {}
            feat_tiles = {}
            aux_tiles = {}

            def emit_st(wi, ws, qwd, g):
                # each of the GSZ concurrent volley matmuls owns a full 2KB
                # PSUM bank (512 f32 cols padded), required by hardware
                st = ps.tile(
                    [P, GSZ, 512], f32, tag="st", name=f"st{wi}_{g}"
                )
                for j in range(GSZ):
                    kc = GSZ * g + j
                    nc.tensor.matmul(
                        st[:, j, :qwd],
                        K_sb[32 * j : 32 * j + CQ, kc * P : (kc + 1) * P],
                        Q_sb[32 * j : 32 * j + CQ, ws : ws + qwd],
                        start=True,
                        stop=True,
                        tile_position=(32 * j, 0),
                    )
                et = ep.tile([P, GSZ, qwd], fp16, tag="e", name=f"e{wi}_{g}")
                nc.scalar.activation(et, st[:, :, :qwd], Exp)
                et_tiles[(wi, g)] = et
                # DVE side of the denominator: elementwise accumulate the exp
                # tiles (2x fp16 DVE mode); the last XPE groups are left for
                # PE ones-matmuls at feat time to balance the two engines.
                if g < NG - xpe:
                    if g == 0:
                        acc = accp.tile(
                            [P, GSZ, qwd], fp16, tag="acc", name=f"acc{wi}"
                        )
                        acc_tiles[wi] = acc
                        nc.vector.tensor_copy(acc, et)
                    else:
                        nc.vector.tensor_add(acc_tiles[wi], acc_tiles[wi], et)

            def emit_fd(wi, ws, qwd, g):
                if g == 0:
                    feat_tiles[wi] = featp.tile(
                        [P, qwd], f32, tag="feat", name=f"feat{wi}",
                        padded_shape=[P, 512],
                    )
                    aux_tiles[wi] = auxp.tile(
                        [P, qwd], f32, tag="aux", name=f"aux{wi}",
                        padded_shape=[P, 512],
                    )
                feat_ps = feat_tiles[wi]
                aux = aux_tiles[wi]
                et = et_tiles.pop((wi, g))
                for j in range(GSZ):
                    kc = GSZ * g + j
                    nc.tensor.matmul(
                        feat_ps,
                        VT[:, kc * P : (kc + 1) * P],
                        et[:, j, :],
                        start=(kc == 0),
                        stop=(kc == KC - 1),
                    )
                if g >= NG - xpe:
                    # PE share of the denominator (all 128 output rows carry
                    # the same key-sum, v1-style)
                    for j in range(GSZ):
                        nc.tensor.matmul(
                            aux,
                            ones_big,
                            et[:, j, :],
                            start=(g == NG - xpe and j == 0),
                            stop=False,
                        )
                if g == NG - 1:
                    acc = acc_tiles.pop(wi)
                    for j in range(GSZ):
                        nc.tensor.matmul(
                            aux,
                            ones_big,
                            acc[:, j, :],
                            start=(xpe == 0 and j == 0),
                            stop=(j == GSZ - 1),
                        )
                    aux_tiles.pop(wi)
                    rec = small.tile([P, qwd], f32, tag="rec", name=f"rec{wi}")
                    nc.vector.reciprocal(rec, aux)
                    o_sb = op.tile([P, qwd], f32, tag="o", name=f"o{wi}")
                    nc.vector.tensor_mul(o_sb, feat_tiles.pop(wi), rec)
                    nc.sync.dma_start(o.ap()[:, ws : ws + qwd], o_sb)

            for idx in range(len(groups) + lag):
                if idx < len(groups):
                    emit_st(*groups[idx])
                if idx >= lag:
                    emit_fd(*groups[idx - lag])

    nc.compile()
    return nc


def _get_runner(xpe=None, lag=None):
    """Build the Bass module once and wrap it in a cached sharded jax callable.

    Mirrors concourse.bass2jax.run_bass_via_pjrt (the @via_axon execution
    path) but caches the jitted executable so repeated kernel() calls do not
    re-trace/re-compile.
    """
    key = ("runner", xpe, lag)
    if key in _CACHE:
        return _CACHE[key]

    import jax
    from jax.experimental.shard_map import shard_map
    from jax.sharding import Mesh, PartitionSpec

    from concourse import bass2jax, mybir as _mybir

    bass2jax.install_neuronx_cc_hook()
    nc = _build(xpe, lag)

    partition_name = nc.partition_id_tensor.name if nc.partition_id_tensor else None
    in_names, out_names, out_avals = [], [], []
    for alloc in nc.m.functions[0].allocations:
        if not isinstance(alloc, _mybir.MemoryLocationSet):
            continue
        name = alloc.memorylocations[0].name
        if alloc.kind == "ExternalInput":
            if name != partition_name:
                in_names.append(name)
        elif alloc.kind == "ExternalOutput":
            out_names.append(name)
            out_avals.append(
                jax.core.ShapedArray(
                    tuple(alloc.tensor_shape), _mybir.dt.np(alloc.dtype)
                )
            )
    n_params = len(in_names)
    all_in_names = in_names + out_names
    if partition_name is not None:
        all_in_names.append(partition_name)
    donate = tuple(range(n_params, n_params + len(out_names)))

    def _body(*args):
        operands = list(args)
        if partition_name is not None:
            operands.append(bass2jax.partition_id_tensor())
        outs = bass2jax._bass_exec_p.bind(
            *operands,
            out_avals=tuple(out_avals),
            in_names=tuple(all_in_names),
            out_names=tuple(out_names),
            lowering_input_output_aliases=(),
            sim_require_finite=True,
            sim_require_nnan=True,
            nc=nc,
        )
        return tuple(outs)

    devices = jax.devices()[:8]
    mesh = Mesh(np.asarray(devices), ("core",))
    in_specs = (PartitionSpec("core"),) * (n_params + len(out_names))
    out_specs = (PartitionSpec("core"),) * len(out_names)
    smapped = shard_map(
        _body, mesh=mesh, in_specs=in_specs, out_specs=out_specs, check_rep=False
    )
    sharded = jax.jit(smapped, donate_argnums=donate, keep_unused=True)

    out_shapes = [tuple(a.shape) for a in out_avals]
    out_dtypes = [a.dtype for a in out_avals]
    runner = {
        "fn": sharded,
        "smapped": smapped,
        "n_params": n_params,
        "in_names": in_names,
        "out_names": out_names,
        "out_shapes": out_shapes,
        "out_dtypes": out_dtypes,
        "nc": nc,
    }
    _CACHE[key] = runner
    return runner


def _run(in_maps):
    r = _get_runner()
    concat_in = [
        np.concatenate([np.asarray(m[name]) for m in in_maps], axis=0)
        for name in r["in_names"]
    ]
    concat_zeros = [
        np.zeros((8 * s[0], *s[1:]), d)
        for s, d in zip(r["out_shapes"], r["out_dtypes"])
    ]
    out_arrs = r["fn"](*concat_in, *concat_zeros)
    return [
        {
            name: np.asarray(out_arrs[i]).reshape(8, *r["out_shapes"][i])[c]
            for i, name in enumerate(r["out_names"])
        }
        for c in range(8)
    ]


def _make_in_maps(x, y, w_q, w_k, w_v):
    x = np.ascontiguousarray(np.asarray(x, dtype=np.float32))
    y = np.ascontiguousarray(np.asarray(y, dtype=np.float32))
    bz, c, h, w = x.shape
    n = h * w
    xf = x.reshape(bz, c, n)
    yf = y.reshape(bz, c, n)
    wqT = np.asarray(w_q, dtype=np.float32).T  # [c, cq]
    wkT = np.asarray(w_k, dtype=np.float32).T
    z = np.zeros((c, 32 - CQ), np.float32)
    wq2 = np.ascontiguousarray(
        np.concatenate([wqT, z, wqT, z, wqT, z, wqT], axis=1)
    )  # [c, 112]
    wk2 = np.ascontiguousarray(np.concatenate([wkT, z, wkT, z, wkT, z, wkT], axis=1))
    wvT = np.ascontiguousarray(np.asarray(w_v, dtype=np.float32).T)  # [c, c]
    in_maps = []
    for cid in range(8):
        b, qb = divmod(cid, 4)
        in_maps.append(
            {
                "y": np.ascontiguousarray(yf[b]),
                "yq": np.ascontiguousarray(yf[b][:, qb * NQ : (qb + 1) * NQ]),
                "x": np.ascontiguousarray(xf[b]),
                "wq": wq2,
                "wk": wk2,
                "wv": wvT,
            }
        )
    return in_maps


def kernel(x, y, w_q, w_k, w_v):
    bz, c, h, w = np.asarray(x).shape
    n = h * w
    results = _run(_make_in_maps(x, y, w_q, w_k, w_v))
    feat = np.empty((bz, c, n), dtype=np.float32)
    for cid in range(8):
        b, qb = divmod(cid, 4)
        feat[b][:, qb * NQ : (qb + 1) * NQ] = results[cid]["o"]
    return feat.reshape(bz, c, h, w)
